# revision 17
# baseline (speedup 1.0000x reference)
"""Trainium2 Bass kernel for nn_MultiHeadAttention (B=8192, D=1024, 16 heads
used only via the softmax scale 1/8).

Zero-collective formulation (8 NeuronCores, row-sharded):
  - Rows (batch axis) of the attention output are sharded: core c owns rows
    [c*1024, (c+1)*1024). Every core receives the FULL x as an input, so no
    on-device communication is needed at all (arming ncfw collectives was
    measured to slow EVERY subsequent PE instruction by ~25-50 ns).
  - K is never materialized:  E^T = x @ G^T (+ Q.bk, constant per query —
    cancels in softmax), where G^T[d,i] = sum_o Wk[o,d] Q^T[o,i] is a small
    local matmul whose stationary is Wk in NATURAL layout (no transpose).
  - V is never materialized:  attn@V = (PX^T)^T @ Wv^T + bv, where
    PX^T[d,i] = sum_j x[j,d] P^T[j,i] uses x rows in NATURAL layout as the
    stationary — only the epilogue needs Wv^T.
  - The full x^T (fp16) is PE-transposed once into DRAM (4 groups so phase 2
    can start on group 0 early) and streamed back per j-block.
  - Attention runs in the transposed-energy ("E^T") layout:
        E^T[j, i] = sum_d x^T[d, j] * G^T[d, i]       (fp16 operands)
        P^T = exp(E^T * 0.125)                        (no max subtraction)
        PX^T[d, i] = sum_j x[j, d] * P^T[j, i]        (bf16)
        s[i] = sum_j P^T[j, i]  (ones-stationary matmul, transposed at end)
        out = (PX^T)^T @ Wv^T / s + bv
"""

import sys

sys.path.insert(0, "/opt/trn_rl_repo")

import numpy as np

import concourse.bass as bass  # noqa: F401
import concourse.tile as tile
from concourse import bacc, mybir
from concourse.bass_utils import run_bass_kernel_spmd
from concourse.masks import make_identity

B = 8192
D = 1024
P = 128
NCORES = 8
R = B // NCORES  # 1024 rows per core
JBLK = 512  # j-block streamed per iteration
NJB = B // JBLK  # 16
NG = 4  # x^T DRAM groups (finer dep granularity for overlap)
GJB = NJB // NG  # j-blocks per group
DO = D // P  # 8 feature chunks of 128
IC = R // P  # 8 row chunks of 128 per core
F32 = mybir.dt.float32
F32R = mybir.dt.float32r
BF16 = mybir.dt.bfloat16
F16 = mybir.dt.float16
AF = mybir.ActivationFunctionType
ALU = mybir.AluOpType
SCALE = 0.125  # 1/sqrt(head_dim=64)


def build_program():
    nc = bacc.Bacc(
        "TRN2", target_bir_lowering=False, debug=False, num_devices=NCORES
    )
    # f32r is bit-identical to f32 (np.float32 feeds it); declaring the
    # matmul-bound inputs as f32r keeps their DMAs cast-free
    x_loc = nc.dram_tensor("x_loc", [R, D], F32R, kind="ExternalInput").ap()
    x_all = nc.dram_tensor("x", [B, D], F32R, kind="ExternalInput").ap()
    w_q = nc.dram_tensor("Wq", [D, D], F32R, kind="ExternalInput").ap()
    w_k = nc.dram_tensor("Wk", [D, D], F32R, kind="ExternalInput").ap()
    w_v = nc.dram_tensor("Wv", [D, D], F32R, kind="ExternalInput").ap()
    b_q = nc.dram_tensor("bq", [D], F32, kind="ExternalInput").ap()
    b_v = nc.dram_tensor("bv", [D], F32, kind="ExternalInput").ap()
    out_loc = nc.dram_tensor("out_loc", [R, D], F32, kind="ExternalOutput").ap()

    # full x^T fp16 scratch, split in NG groups of GJB j-blocks
    xt16 = [
        nc.dram_tensor(f"xt16_{g}", [DO, P, GJB * JBLK], F16) for g in range(NG)
    ]

    with tile.TileContext(nc) as tc:
        _body(nc, tc, x_loc, x_all, w_q, w_k, w_v, b_q, b_v, out_loc, xt16)
    nc.compile()
    return nc


def _transpose_block(nc, tp_psum, identity, src_sb, dst, dd, col_off, drain_eng):
    """PE-transpose one [128, 128] block of src into dst[:, dd, col_off:+128],
    draining (and dtype-casting) the PSUM through scalar or vector."""
    tp = tp_psum.tile([P, P], F32R, tag="tp")
    nc.tensor.transpose(tp, src_sb[:, dd * P : (dd + 1) * P], identity)
    if drain_eng == "scalar":
        nc.scalar.activation(dst[:, dd, col_off : col_off + P], tp, AF.Identity)
    else:
        nc.vector.tensor_copy(out=dst[:, dd, col_off : col_off + P], in_=tp)


def _body(nc, tc, x_loc, x_all, w_q, w_k, w_v, b_q, b_v, out_loc, xt16):
    from contextlib import ExitStack

    outer = ExitStack()
    outer.__enter__()
    # ---- persistent pools (whole kernel) ----
    const_pool = outer.enter_context(tc.tile_pool(name="const", bufs=1))
    identity_f32 = const_pool.tile([P, P], F32)
    make_identity(nc, identity_f32)
    identity = const_pool.tile([P, P], F32R)  # f32r for 1.5 cyc/row transposes
    nc.vector.tensor_copy(out=identity, in_=identity_f32)
    ones_f32 = const_pool.tile([P, 2], F32)
    nc.vector.memset(ones_f32, 1.0)
    ones = const_pool.tile([P, 2], BF16)
    nc.vector.tensor_copy(out=ones, in_=ones_f32)
    bq_sb = const_pool.tile([P, DO], F32)
    nc.sync.dma_start(bq_sb, b_q.rearrange("(oo p) -> p oo", p=P))
    ones_row = const_pool.tile([1, P], F32)
    nc.vector.memset(ones_row, 1.0)
    # broadcast bv across all 128 partitions with a K=1 matmul
    bv_bc = const_pool.tile([P, D], F32)
    nc.sync.dma_start(bv_bc[0:1, :], b_v[None, :])
    with tc.tile_pool(name="bv_psum", bufs=2, space="PSUM") as bvp:
        for oh in range(2):
            pt = bvp.tile([P, 512], F32, tag="bvp")
            nc.tensor.matmul(
                pt,
                ones_row,
                bv_bc[0:1, oh * 512 : (oh + 1) * 512],
                start=True,
                stop=True,
            )
            nc.vector.tensor_copy(out=bv_bc[:, oh * 512 : (oh + 1) * 512], in_=pt)

    gt_pool = outer.enter_context(tc.tile_pool(name="gt", bufs=1))
    gt = gt_pool.tile([P, DO, R], F16)  # G^T = (Q Wk)^T: [d_in, d, i] (2 MB)
    wvt_pool = outer.enter_context(tc.tile_pool(name="wvt", bufs=1))
    wvt = wvt_pool.tile([P, DO, D], BF16)  # Wv^T for the epilogue (2 MB)

    sums_pool = outer.enter_context(tc.tile_pool(name="sums", bufs=1))
    sums_sb = sums_pool.tile([2, R], F32)  # row of exp-sums (free axis = i)
    rsum = sums_pool.tile([P, 2 * IC], F32)  # transposed to per-partition
    pxb_pool = outer.enter_context(tc.tile_pool(name="pxb", bufs=1))
    pxb = pxb_pool.tile([P, DO, R], BF16)  # PX^T, bf16 for the epilogue (2 MB)

    # =========================================================
    # Phase 1a: W transposes, Q^T (local rows, biased), G^T
    # =========================================================
    with ExitStack() as p1:
        wt_pool = p1.enter_context(tc.tile_pool(name="wt", bufs=1))
        wqt = wt_pool.tile([P, DO, D], F32R)  # Wq^T (4 MB)
        wkn = wt_pool.tile([P, DO, D], F32R)  # Wk NATURAL rows (4 MB)
        xt = wt_pool.tile([P, DO, R], F32R)  # local x^T (4 MB)
        qt = wt_pool.tile([P, DO, R], F32R)  # Q^T f32r (4 MB)

        row_pool = p1.enter_context(tc.tile_pool(name="rows", bufs=3))
        tp_psum = p1.enter_context(tc.tile_pool(name="tp_ps", bufs=2, space="PSUM"))
        mm_psum = p1.enter_context(tc.tile_pool(name="mm_ps", bufs=4, space="PSUM"))

        for oo in range(DO):
            nc.sync.dma_start(wkn[:, oo, :], w_k[oo * P : (oo + 1) * P, :])
        for wt_sb, w_dram in ((wqt, w_q), (wvt, w_v)):
            for oo in range(DO):
                wrow = row_pool.tile([P, D], F32R, tag="row")
                nc.sync.dma_start(wrow, w_dram[oo * P : (oo + 1) * P, :])
                for dd in range(DO):
                    _transpose_block(
                        nc, tp_psum, identity, wrow, wt_sb, dd, oo * P,
                        "scalar" if dd % 2 else "vector",
                    )
        for jj in range(IC):
            xrow = row_pool.tile([P, D], F32R, tag="row")
            nc.sync.dma_start(xrow, x_loc[jj * P : (jj + 1) * P, :])
            for dd in range(DO):
                _transpose_block(
                    nc, tp_psum, identity, xrow, xt, dd, jj * P,
                    "scalar" if dd % 2 else "vector",
                )

        # Q^T = Wq x^T + bq (local rows)
        for ih in range(2):
            for oo in range(DO):
                pq = mm_psum.tile([P, JBLK], F32, tag="mm")
                for dd in range(DO):
                    nc.tensor.matmul(
                        pq,
                        (wqt[:, dd, oo * P : (oo + 1) * P]),
                        (xt[:, dd, ih * JBLK : (ih + 1) * JBLK]),
                        start=(dd == 0),
                        stop=(dd == DO - 1),
                    )
                nc.scalar.activation(
                    qt[:, oo, ih * JBLK : (ih + 1) * JBLK],
                    pq,
                    AF.Identity,
                    bias=bq_sb[:, oo : oo + 1],
                )
        # G^T[d, i] = sum_o Wk[o, d] Q^T[o, i]   (bk drops out of softmax)
        for dd in range(DO):
            for ih in range(2):
                pg = mm_psum.tile([P, JBLK], F32, tag="mm")
                for oo in range(DO):
                    nc.tensor.matmul(
                        pg,
                        (wkn[:, oo, dd * P : (dd + 1) * P]),
                        (qt[:, oo, ih * JBLK : (ih + 1) * JBLK]),
                        start=(oo == 0),
                        stop=(oo == DO - 1),
                    )
                nc.scalar.activation(
                    gt[:, dd, ih * JBLK : (ih + 1) * JBLK], pg, AF.Identity
                )

    # =========================================================
    # Phase 1b: full x^T -> fp16 DRAM, in NG groups
    # =========================================================
    with ExitStack() as p1b:
        row_pool = p1b.enter_context(tc.tile_pool(name="rows2", bufs=3))
        xst_pool = p1b.enter_context(tc.tile_pool(name="xst", bufs=2))
        tp_psum = p1b.enter_context(tc.tile_pool(name="tp2_ps", bufs=2, space="PSUM"))
        for g in range(NG):
            for bb in range(GJB):  # 512-row blocks within the group
                xst = xst_pool.tile([P, DO, JBLK], F16, tag="xst")
                for jj in range(JBLK // P):
                    xrow = row_pool.tile([P, D], F32R, tag="row")
                    r0 = (g * GJB + bb) * JBLK + jj * P
                    nc.sync.dma_start(xrow, x_all[r0 : r0 + P, :])
                    for dd in range(DO):
                        _transpose_block(
                            nc, tp_psum, identity, xrow, xst, dd, jj * P,
                            "scalar" if dd % 2 else "vector",
                        )
                for dd in range(DO):
                    nc.sync.dma_start(
                        xt16[g][dd, :, bb * JBLK : (bb + 1) * JBLK],
                        xst[:, dd, :],
                    )

    # =========================================================
    # Phase 2: streamed attention in E^T layout over 16 j-blocks
    # =========================================================
    with ExitStack() as p2:
        px_pool = p2.enter_context(tc.tile_pool(name="pxacc", bufs=1))
        pxacc = px_pool.tile([P, DO, R], F32)  # PX^T accumulator (4 MB)
        xtb_pool = p2.enter_context(tc.tile_pool(name="xtb", bufs=3))
        xnf_pool = p2.enter_context(tc.tile_pool(name="xnf", bufs=2))
        xn_pool = p2.enter_context(tc.tile_pool(name="xn", bufs=3))
        pt_pool = p2.enter_context(tc.tile_pool(name="ptb", bufs=3))
        e_psum = p2.enter_context(tc.tile_pool(name="e_ps", bufs=3, space="PSUM"))
        px_psum = p2.enter_context(tc.tile_pool(name="px_ps", bufs=3, space="PSUM"))
        s_psum = p2.enter_context(tc.tile_pool(name="s_ps", bufs=1, space="PSUM"))

        s_ps = [
            s_psum.tile([2, JBLK], F32, tag=f"sps{ih}", name=f"sps{ih}")
            for ih in range(2)
        ]

        for jb in range(NJB):
            g, bb = jb // GJB, jb % GJB
            xtjb = xtb_pool.tile([P, DO, JBLK], F16, tag="xtb")
            for dd in range(DO):
                nc.sync.dma_start(
                    xtjb[:, dd, :], xt16[g][dd, :, bb * JBLK : (bb + 1) * JBLK]
                )
            # x rows natural (stationary for PX^T), cast f32r -> bf16 on DVE
            xnf = xnf_pool.tile([P, JBLK // P, D], F32R, tag="xnf")
            nc.sync.dma_start(
                xnf,
                x_all[jb * JBLK : (jb + 1) * JBLK, :].rearrange(
                    "(jj p) o -> p jj o", p=P
                ),
            )
            xn = xn_pool.tile([P, JBLK // P, D], BF16, tag="xn")
            nc.vector.tensor_copy(out=xn, in_=xnf)

            # energy E^T and probabilities P^T for this j-block
            ptb = pt_pool.tile([P, JBLK // P, R], BF16, tag="ptb")
            for jj in range(JBLK // P):
                pe_h = [
                    e_psum.tile([P, JBLK], F32, tag="pe", name="pe")
                    for _ in range(2)
                ]
                for dd in range(DO):
                    for ih in range(2):
                        nc.tensor.matmul(
                            pe_h[ih],
                            (xtjb[:, dd, jj * P : (jj + 1) * P]),
                            (gt[:, dd, ih * JBLK : (ih + 1) * JBLK]),
                            start=(dd == 0),
                            stop=(dd == DO - 1),
                        )
                for ih in range(2):
                    nc.scalar.activation(
                        ptb[:, jj, ih * JBLK : (ih + 1) * JBLK],
                        pe_h[ih],
                        AF.Exp,
                        scale=SCALE,
                    )
                # exp-sums ride the same moving stream (ones stationary)
                for ih in range(2):
                    nc.tensor.matmul(
                        s_ps[ih],
                        ones,
                        (ptb[:, jj, ih * JBLK : (ih + 1) * JBLK]),
                        start=(jb == 0 and jj == 0),
                        stop=(jb == NJB - 1 and jj == JBLK // P - 1),
                    )
            # PX^T += x_nat.T-contracted with P^T
            for dd in range(DO):
                for ih in range(2):
                    pp = px_psum.tile([P, JBLK], F32, tag="pp")
                    for jj in range(JBLK // P):
                        nc.tensor.matmul(
                            pp,
                            (xn[:, jj, dd * P : (dd + 1) * P]),
                            (ptb[:, jj, ih * JBLK : (ih + 1) * JBLK]),
                            start=(jj == 0),
                            stop=(jj == JBLK // P - 1),
                        )
                    dst = pxacc[:, dd, ih * JBLK : (ih + 1) * JBLK]
                    if jb == 0:
                        nc.vector.tensor_copy(out=dst, in_=pp)
                    else:
                        nc.vector.tensor_tensor(dst, pp, dst, ALU.add)

        # ---- sums: drain the psum accumulators to SBUF (frees the banks);
        # cast PX^T to bf16 for the epilogue before this pool scope closes ----
        for ih in range(2):
            nc.vector.tensor_copy(
                out=sums_sb[:, ih * JBLK : (ih + 1) * JBLK], in_=s_ps[ih]
            )
        nc.vector.tensor_copy(out=pxb, in_=pxacc)

    # =========================================================
    # Phase 3: transpose sums to per-partition, reciprocal, then
    # out = (PX^T)^T @ Wv^T / s + bv
    # =========================================================
    with ExitStack() as p3:
        st_ps = p3.enter_context(tc.tile_pool(name="st_ps", bufs=2, space="PSUM"))
        # (pxb was cast at the end of phase 2)
        for ic in range(IC):
            stp = st_ps.tile([P, 2], F32, tag="stp")
            nc.tensor.transpose(
                stp,
                sums_sb[:, ic * P : (ic + 1) * P],
                identity_f32[0:2, 0:2],
            )
            nc.vector.tensor_copy(out=rsum[:, 2 * ic : 2 * ic + 2], in_=stp)
        nc.vector.reciprocal(rsum, rsum)
        fin_pool = p3.enter_context(tc.tile_pool(name="fin", bufs=2))
        o_psum = p3.enter_context(tc.tile_pool(name="o_ps", bufs=3, space="PSUM"))
        for ic in range(IC):
            ofin = fin_pool.tile([P, D], F32, tag="ofin")
            for oh in range(2):
                po = o_psum.tile([P, 512], F32, tag="po")
                for dd in range(DO):
                    nc.tensor.matmul(
                        po,
                        (pxb[:, dd, ic * P : (ic + 1) * P]),
                        (wvt[:, dd, oh * 512 : (oh + 1) * 512]),
                        start=(dd == 0),
                        stop=(dd == DO - 1),
                    )
                nc.vector.tensor_scalar_mul(
                    ofin[:, oh * 512 : (oh + 1) * 512],
                    po,
                    rsum[:, 2 * ic : 2 * ic + 1],
                )
            nc.vector.tensor_tensor(ofin, ofin, bv_bc, ALU.add)
            nc.sync.dma_start(out_loc[ic * P : (ic + 1) * P, :], ofin)

    outer.close()


_NC_CACHE = None


def _get_program():
    global _NC_CACHE
    if _NC_CACHE is None:
        _NC_CACHE = build_program()
    return _NC_CACHE


def _run(inputs, trace=False):
    nc = _get_program()
    x = np.ascontiguousarray(np.asarray(inputs["x"], dtype=np.float32))
    common = {
        k: np.ascontiguousarray(np.asarray(inputs[k], dtype=np.float32))
        for k in ("Wq", "Wk", "Wv", "bq", "bv")
    }
    in_maps = [
        {
            "x": x,
            "x_loc": np.ascontiguousarray(x[c * R : (c + 1) * R]),
            **common,
        }
        for c in range(NCORES)
    ]
    res = run_bass_kernel_spmd(
        nc, in_maps, core_ids=list(range(NCORES)), trace=trace
    )
    out = np.concatenate([res.results[c]["out_loc"] for c in range(NCORES)], axis=0)
    return out.reshape(B, D, 1).astype(np.float32), res


def kernel(**inputs):
    out, _ = _run(inputs, trace=False)
    return out
